# revision 14
# baseline (speedup 1.0000x reference)
"""Trainium2 Bass kernel for cross-modal channel-attention fusion (CCDPA).

Math (per batch b):
  pooled[c,m,d] = mean_{w,h} x_m[b,c,d,w,h]
  q = Wq @ pooled[:,0,:] + bq ; k_m = Wk @ pooled[:,m,:] + bk
  a[c,m] = softmax_m(q[c]·k_m[c] / sqrt(D))
  out[b,o,s] = sum_m a[o,m] * (Wc[m] @ x_m[b,:,s] + bc[m,o])
             = sum_m (a[o,m]*Wc[m,o,:]) @ x_m[b,:,s]  + sum_m a[o,m]*bc[m,o]

Sharding: 8 cores = (batch b = p//2) x (d-half = p%2).

Layout: the host repacks each core's 64 MiB shard into two channel-half
tensors xh[ci] of shape [128, U*4*nw] where unit u holds the 4 modality
blocks for s-chunk u ([128, 4*nw], nw=512). One 1 MiB DMA then delivers a
full GEMM contraction set for 512 output columns, so pass-2 tile liveness
is tiny and prefetch pipelines at unit granularity.

Schedule: pass 1 streams ci=0 units then ci=1 units, reducing pooling sums
per (m, unit). The pairwise AllGather for channel-half 0 fires at the
half-way point and its attention chain (PE transposes + q/k matmuls + ACT
copies mid-stream, DVE softmax/weff tail after the loop) completes just
after pass-1 ends, so oi=0 GEMMs start immediately; half-1's collective
latency is bridged by bf16-cached unit GEMMs and re-read prefetch. The
per-(d, sub-unit) pooling sums are contracted directly against augmented
q/k weight matrices with duplicated rows (host-folded), which also absorb
the 1/(W*H) mean, the 1/sqrt(D) logit scale and the q/k biases via a
ones-row.

GEMMs run in fp32r (full PE rate at moving dim 512) for re-read units and
bf16 for cached units; 10 units (20 MiB fp32-equivalent) stay cached in
SBUF as bf16 between the passes.
"""

from contextlib import ExitStack

import numpy as np

import concourse.bacc as bacc
import concourse.bass as bass
import concourse.mybir as mybir
import concourse.tile as tile
from concourse.bass_utils import run_bass_kernel_spmd

F32 = mybir.dt.float32
BF16 = mybir.dt.bfloat16

B, C, D, W, H = 4, 256, 32, 32, 32
NCORES = 8
DHALF = D // 2  # d-slices per core
WH = W * H  # spatial elements per d-slice
S = DHALF * WH  # free elements per core shard

MM_DT = mybir.dt.float32r  # fp32 bits, full PE rate at moving dim >= 256

NW = 512  # matmul moving-dim chunk = unit width
NCACHE = 12  # units cached in SBUF as bf16 between the passes
STREAM_BUFS = 8
DD = 32  # attention feature dim (= D)


def _cfg(wh, dhalf):
    nw = min(NW, wh)
    u = dhalf * wh // nw  # units per channel-half
    ncache = min(NCACHE, max(u - 2, 0))
    return nw, u, ncache


def _emit_program(nc, wh=WH, dhalf=DHALF, mm_dt=MM_DT):
    """Emit the SPMD per-core program. Identical on all 8 cores; per-core
    behavior comes only from per-core input data."""
    f32 = F32
    s = dhalf * wh
    dd = 2 * dhalf  # attention feature dim (= D for this config)
    nw, U, ncache = _cfg(wh, dhalf)
    ucache0 = U - ncache  # units [ucache0, U) are cached
    uw = 4 * nw  # unit width (4 modality blocks)
    AX = mybir.AxisListType.X
    AF = mybir.ActivationFunctionType

    xh = [
        nc.dram_tensor(f"xh{ci}", [128, U * uw], f32, kind="ExternalInput")
        for ci in range(2)
    ]
    wq2_d = nc.dram_tensor("wq2", [2 * U + 1, dd], f32, kind="ExternalInput")
    wk2_d = nc.dram_tensor("wk2", [2 * U + 1, dd], f32, kind="ExternalInput")
    wc_d = nc.dram_tensor("wc", [4, C, C], f32, kind="ExternalInput")
    bcT_d = nc.dram_tensor("bcT", [C, 4], f32, kind="ExternalInput")
    id_d = nc.dram_tensor("ident", [128, 128], f32, kind="ExternalInput")
    out_d = nc.dram_tensor("out", [C, s], BF16, kind="ExternalOutput")

    with tile.TileContext(nc) as tc, ExitStack() as ctx:
        const = ctx.enter_context(tc.tile_pool(name="const", bufs=1))
        stream = ctx.enter_context(tc.tile_pool(name="stream", bufs=STREAM_BUFS))
        outp = ctx.enter_context(tc.tile_pool(name="outp", bufs=4))
        attn = ctx.enter_context(tc.tile_pool(name="attn", bufs=1))
        scr = ctx.enter_context(tc.tile_pool(name="scr", bufs=2))
        psA = ctx.enter_context(tc.tile_pool(name="psA", bufs=3, space="PSUM"))
        psM = ctx.enter_context(tc.tile_pool(name="psM", bufs=5, space="PSUM"))
        dramp = ctx.enter_context(tc.tile_pool(name="dramp", bufs=1, space="DRAM"))

        # ---- constant loads (off critical path) ----
        ident = const.tile([128, 128], f32, tag="ident", name="ident")
        nc.sync.dma_start(out=ident[:], in_=id_d[:])
        wq2 = const.tile([2 * U + 1, dd], f32, tag="wq2", name="wq2")
        nc.sync.dma_start(out=wq2[:], in_=wq2_d[:])
        wk2 = const.tile([2 * U + 1, dd], f32, tag="wk2", name="wk2")
        nc.sync.dma_start(out=wk2[:], in_=wk2_d[:])
        wc_sb = []
        for oi in range(2):
            t = const.tile([128, 4 * C], f32, tag=f"wc{oi}", name=f"wc{oi}")
            for m in range(4):
                nc.sync.dma_start(
                    out=t[:, m * C : (m + 1) * C],
                    in_=wc_d[m, oi * 128 : (oi + 1) * 128, :],
                )
            wc_sb.append(t)
        bc_sb = []
        for oi in range(2):
            t = const.tile([128, 4], f32, tag=f"bc{oi}", name=f"bc{oi}")
            nc.sync.dma_start(out=t[:], in_=bcT_d[oi * 128 : (oi + 1) * 128, :])
            bc_sb.append(t)

        # persistent attention-state tiles
        praw = [
            attn.tile([128, 4 * U], f32, tag=f"praw{ci}", name=f"praw{ci}")
            for ci in range(2)
        ]
        pooled = [
            attn.tile([128, 4 * 2 * U], f32, tag=f"pool{k}", name=f"pool{k}")
            for k in range(2)
        ]
        ptaug = [
            attn.tile([2 * U + 1, C], f32, tag=f"pt{m}", name=f"pt{m}")
            for m in range(4)
        ]
        qc = [attn.tile([128, dd], f32, tag=f"qc{k}", name=f"qc{k}") for k in range(2)]
        kcs = [
            [attn.tile([128, dd], f32, tag=f"kc{m}_{k}", name=f"kc{m}_{k}") for k in range(2)]
            for m in range(4)
        ]
        xc = [
            attn.tile([128, max(ncache, 1) * uw], BF16, tag=f"xcache{ci}", name=f"xcache{ci}")
            for ci in range(2)
        ]
        wt_sb = [
            attn.tile([128, 4 * C], mm_dt, tag=f"wt{ci}", name=f"wt{ci}")
            for ci in range(2)
        ]
        wtb_sb = [
            attn.tile([128, 4 * C], BF16, tag=f"wtb{ci}", name=f"wtb{ci}")
            for ci in range(2)
        ]
        for m in range(4):
            nc.vector.memset(ptaug[m][:], 1.0)

        cc_in = [
            dramp.tile([128, 4 * U], f32, tag=f"cc_in{ci}", name=f"cc_in{ci}")
            for ci in range(2)
        ]
        cc_out = [
            dramp.tile([256, 4 * U], f32, tag=f"cc_out{ci}", name=f"cc_out{ci}")
            for ci in range(2)
        ]

        def chain_head(k):
            """Readback + transposes + q/k matmuls + psum->sbuf copies for
            channel-half k. PE + ACT only (no DVE: pass-1 reduces own DVE)."""
            for hh in range(2):
                eng = nc.scalar if hh == 0 else nc.gpsimd
                for m in range(4):
                    eng.dma_start(
                        out=pooled[k][:, m * 2 * U + hh * U : m * 2 * U + (hh + 1) * U],
                        in_=cc_out[k][
                            hh * 128 : (hh + 1) * 128, m * U : (m + 1) * U
                        ],
                    )
            for m in range(4):
                # whole [128, 2U] m-block in one transpose: PSUM result and
                # the ptaug copy both start at partition 0 (walrus requires
                # PSUMPartition == 0, and engines cannot shift partitions)
                pst = psA.tile([2 * U, 128], f32, tag="att", name="att")
                nc.tensor.transpose(
                    pst[:],
                    pooled[k][:, m * 2 * U : (m + 1) * 2 * U],
                    ident[:],
                )
                nc.scalar.activation(
                    ptaug[m][0 : 2 * U, k * 128 : (k + 1) * 128],
                    pst[:],
                    AF.Copy,
                )
            psq = psA.tile([128, dd], f32, tag="att", name="att")
            nc.tensor.matmul(
                psq[:], lhsT=ptaug[0][:, k * 128 : (k + 1) * 128], rhs=wq2[:],
                start=True, stop=True,
            )
            nc.scalar.activation(qc[k][:], psq[:], AF.Copy)
            for m in range(4):
                psk = psA.tile([128, dd], f32, tag="att", name="att")
                nc.tensor.matmul(
                    psk[:], lhsT=ptaug[m][:, k * 128 : (k + 1) * 128], rhs=wk2[:],
                    start=True, stop=True,
                )
                nc.scalar.activation(kcs[m][k][:], psk[:], AF.Copy)

        def chain_tail(k):
            """Logits + softmax + scaled weights for output-half oi=k.
            DVE-heavy; emitted only after all pass-1 reduces."""
            lg = attn.tile([128, 4], f32, tag=f"lg{k}", name=f"lg{k}")
            for m in range(4):
                sc = scr.tile([128, dd], f32, tag="ttr", name="ttr")
                nc.vector.tensor_mul(sc[:], qc[k][:], kcs[m][k][:])
                nc.vector.reduce_sum(out=lg[:, m : m + 1], in_=sc[:], axis=AX)
            # logits are O(1e-4) here (weights scaled by 0.02), so exp() is
            # safe without the max-subtraction step
            ex = attn.tile([128, 4], f32, tag=f"ex{k}", name=f"ex{k}")
            nc.scalar.activation(ex[:], lg[:], AF.Exp)
            sm = attn.tile([128, 1], f32, tag=f"sm{k}", name=f"sm{k}")
            nc.vector.reduce_sum(out=sm[:], in_=ex[:], axis=AX)
            rc = attn.tile([128, 1], f32, tag=f"rc{k}", name=f"rc{k}")
            nc.vector.reciprocal(out=rc[:], in_=sm[:])
            at = attn.tile([128, 4], f32, tag=f"a{k}", name=f"a{k}")
            nc.vector.tensor_scalar_mul(out=at[:], in0=ex[:], scalar1=rc[:])
            # weff = a * wc rows (oi = k), effective bias
            weff = attn.tile([128, 4 * C], f32, tag=f"weff{k}", name=f"weff{k}")
            for m in range(4):
                nc.vector.tensor_scalar_mul(
                    out=weff[:, m * C : (m + 1) * C],
                    in0=wc_sb[k][:, m * C : (m + 1) * C],
                    scalar1=at[:, m : m + 1],
                )
            bt = scr.tile([128, 4], f32, tag="btmp", name="btmp")
            be = attn.tile([128, 1], f32, tag=f"beff{k}", name=f"beff{k}")
            nc.vector.tensor_mul(bt[:], at[:], bc_sb[k][:])
            nc.vector.reduce_sum(out=be[:], in_=bt[:], axis=AX)
            # wt[ci][:, m*C + k*128 : +128] = weff[:, m*C + ci*128]^T
            for m in range(4):
                for ci in range(2):
                    psw = psA.tile([128, 128], f32, tag="att", name="att")
                    nc.tensor.transpose(
                        psw[:],
                        weff[:, m * C + ci * 128 : m * C + (ci + 1) * 128],
                        ident[:],
                    )
                    nc.vector.tensor_copy(
                        wt_sb[ci][:, m * C + k * 128 : m * C + (k + 1) * 128],
                        psw[:].bitcast(mm_dt),
                    )
                    nc.scalar.activation(
                        wtb_sb[ci][:, m * C + k * 128 : m * C + (k + 1) * 128],
                        psw[:],
                        AF.Copy,
                    )
            return be

        # ---- pass 1: stream ci=0 units, then ci=1 units ----
        for ci in range(2):
            for u in range(U):
                t = stream.tile([128, uw], f32, tag="x1", name="x1")
                nc.sync.dma_start(out=t[:], in_=xh[ci][:, u * uw : (u + 1) * uw])
                for m in range(4):
                    nc.vector.reduce_sum(
                        out=praw[ci][:, m * U + u : m * U + u + 1],
                        in_=t[:, m * nw : (m + 1) * nw],
                        axis=AX,
                    )
                if u >= ucache0:
                    nc.scalar.activation(
                        xc[ci][:, (u - ucache0) * uw : (u - ucache0 + 1) * uw],
                        t[:],
                        AF.Copy,
                    )
            nc.sync.dma_start(out=cc_in[ci][:], in_=praw[ci][:])
            nc.gpsimd.collective_compute(
                "AllGather",
                mybir.AluOpType.bypass,
                replica_groups=[[0, 1], [2, 3], [4, 5], [6, 7]],
                ins=[cc_in[ci].opt()],
                outs=[cc_out[ci].opt()],
            )
            if ci == 0:
                chain_head(0)

        beff0 = chain_tail(0)
        chain_head(1)

        # ---- pass 2 ----
        def mm_group(units, oi, beff, cached):
            """One LDW-group: the oi-half chunks for a list of units."""
            pss = [psM.tile([128, nw], f32, tag="ps", name="ps") for _ in units]
            for m in range(4):
                for ci in range(2):
                    for i, u in enumerate(units):
                        if cached:
                            rhs = xc[ci][
                                :,
                                (u - ucache0) * uw + m * nw : (u - ucache0) * uw + (m + 1) * nw,
                            ]
                            lhsT = wtb_sb[ci][:, m * C + oi * 128 : m * C + (oi + 1) * 128]
                        else:
                            rhs = xt[(ci, u)][:, m * nw : (m + 1) * nw]
                            lhsT = wt_sb[ci][:, m * C + oi * 128 : m * C + (oi + 1) * 128]
                        nc.tensor.matmul(
                            pss[i][:], lhsT=lhsT, rhs=rhs,
                            start=(m == 0 and ci == 0),
                            stop=(m == 3 and ci == 1),
                        )
            ot = outp.tile([128, len(units) * nw], BF16, tag="ot", name="ot")
            for i, u in enumerate(units):
                nc.vector.tensor_scalar_add(
                    out=ot[:, i * nw : (i + 1) * nw], in0=pss[i][:], scalar1=beff[:]
                )
            nc.scalar.dma_start(
                out=out_d[oi * 128 : (oi + 1) * 128, units[0] * nw : units[0] * nw + len(units) * nw],
                in_=ot[:],
            )

        cu = list(range(ucache0, U))
        cpairs = [cu[i : i + 2] for i in range(0, len(cu), 2)]
        ru = list(range(ucache0))
        rpairs = [ru[i : i + 2] for i in range(0, len(ru), 2)]

        # part 1: cached units, oi=0 (bridges the collective + chain latency)
        for pair in cpairs:
            mm_group(pair, 0, beff0, cached=True)

        beff1 = chain_tail(1)

        # part 2: re-read pairs (both oi) with cached oi=1 pairs interleaved
        cq = list(cpairs)
        xt = {}
        for j, pair in enumerate(rpairs):
            for u in pair:
                for ci in range(2):
                    t = stream.tile([128, uw], mm_dt, tag="x1", name="x1")
                    nc.sync.dma_start(
                        out=t[:],
                        in_=xh[ci][:, u * uw : (u + 1) * uw].bitcast(mm_dt),
                    )
                    xt[(ci, u)] = t
            mm_group(pair, 0, beff0, cached=False)
            mm_group(pair, 1, beff1, cached=False)
            if j % 2 == 1 and cq:
                mm_group(cq.pop(0), 1, beff1, cached=True)
        while cq:
            mm_group(cq.pop(0), 1, beff1, cached=True)
    return nc


_CACHED = {}
LAST_RESULTS = None


def _build(wh=WH, dhalf=DHALF, mm_dt=None):
    if mm_dt is None:
        mm_dt = MM_DT
    key = (wh, dhalf, mm_dt)
    if key not in _CACHED:
        nc = bacc.Bacc(
            "TRN2",
            target_bir_lowering=False,
            debug=False,
            enable_asserts=False,
            num_devices=NCORES,
        )
        _emit_program(nc, wh=wh, dhalf=dhalf, mm_dt=mm_dt)
        nc.compile()
        _CACHED[key] = nc
    return _CACHED[key]


def _host_weights(Wq, bq, Wk, bk, bc, wh, dhalf):
    """Fold pooling mean + logit scale into augmented [2U+1, D] q/k weights
    with one row per (d-half, unit) pooling partial and a trailing ones-row
    for the bias."""
    nw, U, _ = _cfg(wh, dhalf)
    d = 2 * dhalf
    scale_q = 1.0 / (wh * np.sqrt(np.float32(d)))
    wqT = (Wq * scale_q).T  # [d, d] row = source dim
    wkT = (Wk / wh).T
    units_per_d = wh // nw
    wq2 = np.zeros((2 * U + 1, d), np.float32)
    wk2 = np.zeros((2 * U + 1, d), np.float32)
    for hh in range(2):
        for u in range(U):
            dg = hh * dhalf + u // units_per_d
            wq2[hh * U + u] = wqT[dg]
            wk2[hh * U + u] = wkT[dg]
    wq2[2 * U] = bq / np.sqrt(np.float32(d))
    wk2[2 * U] = bk
    bcT = np.ascontiguousarray(bc.T).astype(np.float32)
    ident = np.eye(128, dtype=np.float32)
    return wq2, wk2, bcT, ident


def _core_xh(ms, b, h, wh, dhalf):
    """Repack core (b, h)'s shard into the two [128, U*4*nw] unit tensors."""
    nw, U, _ = _cfg(wh, dhalf)
    s = dhalf * wh
    res = []
    for ci in range(2):
        blocks = [
            ms[m][b, ci * 128 : (ci + 1) * 128, h * dhalf : (h + 1) * dhalf]
            .reshape(128, U, nw)
            for m in range(4)
        ]
        arr = np.stack(blocks, axis=2)  # [128, U, 4, nw]
        res.append(np.ascontiguousarray(arr).reshape(128, U * 4 * nw))
    return res


def kernel(m1, m2, m3, m4, Wq, bq, Wk, bk, Wc, bc, **run_kwargs):
    ms = [np.asarray(x, dtype=np.float32) for x in (m1, m2, m3, m4)]
    Wq, bq, Wk, bk, Wc, bc = (
        np.asarray(x, dtype=np.float32) for x in (Wq, bq, Wk, bk, Wc, bc)
    )
    nc = _build()
    wq2, wk2, bcT, ident = _host_weights(Wq, bq, Wk, bk, bc, WH, DHALF)
    in_maps = []
    for p in range(NCORES):
        b, h = divmod(p, 2)
        x0, x1 = _core_xh(ms, b, h, WH, DHALF)
        im = dict(xh0=x0, xh1=x1, wq2=wq2, wk2=wk2, wc=Wc, bcT=bcT, ident=ident)
        in_maps.append(im)
    global LAST_RESULTS
    res = run_bass_kernel_spmd(
        nc, in_maps, core_ids=list(range(NCORES)), **run_kwargs
    )
    LAST_RESULTS = res
    out = np.empty((B, C, D, W, H), np.float32)
    for p in range(NCORES):
        b, h = divmod(p, 2)
        out[b, :, h * DHALF : (h + 1) * DHALF] = (
            res.results[p]["out"].astype(np.float32).reshape(C, DHALF, W, H)
        )
    return out


# revision 17
# speedup vs baseline: 1.0091x; 1.0091x over previous
"""Trainium2 Bass kernel for cross-modal channel-attention fusion (CCDPA).

Math (per batch b):
  pooled[c,m,d] = mean_{w,h} x_m[b,c,d,w,h]
  q = Wq @ pooled[:,0,:] + bq ; k_m = Wk @ pooled[:,m,:] + bk
  a[c,m] = softmax_m(q[c]·k_m[c] / sqrt(D))
  out[b,o,s] = sum_m a[o,m] * (Wc[m] @ x_m[b,:,s] + bc[m,o])
             = sum_m (a[o,m]*Wc[m,o,:]) @ x_m[b,:,s]  + sum_m a[o,m]*bc[m,o]

Sharding: 8 cores = (batch b = p//2) x (d-half = p%2).

Layout: the host repacks each core's 64 MiB shard into two channel-half
tensors xh[ci] of shape [128, U*4*nw] where unit u holds the 4 modality
blocks for s-chunk u ([128, 4*nw], nw=512). One 1 MiB DMA then delivers a
full GEMM contraction set for 512 output columns, so pass-2 tile liveness
is tiny and prefetch pipelines at unit granularity.

Schedule: pass 1 streams ci=0 units then ci=1 units, reducing pooling sums
per (m, unit). The pairwise AllGather for channel-half 0 fires at the
half-way point and its attention chain (PE transposes + q/k matmuls + ACT
copies mid-stream, DVE softmax/weff tail after the loop) completes just
after pass-1 ends, so oi=0 GEMMs start immediately; half-1's collective
latency is bridged by bf16-cached unit GEMMs and re-read prefetch. The
per-(d, sub-unit) pooling sums are contracted directly against augmented
q/k weight matrices with duplicated rows (host-folded), which also absorb
the 1/(W*H) mean, the 1/sqrt(D) logit scale and the q/k biases via a
ones-row.

GEMMs run in fp32r (full PE rate at moving dim 512) for re-read units and
bf16 for cached units; 10 units (20 MiB fp32-equivalent) stay cached in
SBUF as bf16 between the passes.
"""

from contextlib import ExitStack

import numpy as np

import concourse.bacc as bacc
import concourse.bass as bass
import concourse.mybir as mybir
import concourse.tile as tile
from concourse.bass_utils import run_bass_kernel_spmd

F32 = mybir.dt.float32
BF16 = mybir.dt.bfloat16

B, C, D, W, H = 4, 256, 32, 32, 32
NCORES = 8
DHALF = D // 2  # d-slices per core
WH = W * H  # spatial elements per d-slice
S = DHALF * WH  # free elements per core shard

MM_DT = mybir.dt.float32r  # fp32 bits, full PE rate at moving dim >= 256

NW = 512  # matmul moving-dim chunk = unit width
NCACHE = 11  # units cached in SBUF as bf16 between the passes
STREAM_BUFS = 9
DD = 32  # attention feature dim (= D)


def _cfg(wh, dhalf):
    nw = min(NW, wh)
    u = dhalf * wh // nw  # units per channel-half
    ncache = min(NCACHE, max(u - 2, 0))
    return nw, u, ncache


def _emit_program(nc, wh=WH, dhalf=DHALF, mm_dt=MM_DT):
    """Emit the SPMD per-core program. Identical on all 8 cores; per-core
    behavior comes only from per-core input data."""
    f32 = F32
    s = dhalf * wh
    dd = 2 * dhalf  # attention feature dim (= D for this config)
    nw, U, ncache = _cfg(wh, dhalf)
    ucache0 = U - ncache  # units [ucache0, U) are cached
    uw = 4 * nw  # unit width (4 modality blocks)
    AX = mybir.AxisListType.X
    AF = mybir.ActivationFunctionType

    xh = [
        nc.dram_tensor(f"xh{ci}", [128, U * uw], f32, kind="ExternalInput")
        for ci in range(2)
    ]
    wq2_d = nc.dram_tensor("wq2", [2 * U + 1, dd], f32, kind="ExternalInput")
    wk2_d = nc.dram_tensor("wk2", [2 * U + 1, dd], f32, kind="ExternalInput")
    wc_d = nc.dram_tensor("wc", [4, C, C], f32, kind="ExternalInput")
    bcT_d = nc.dram_tensor("bcT", [C, 4], f32, kind="ExternalInput")
    id_d = nc.dram_tensor("ident", [128, 128], f32, kind="ExternalInput")
    out_d = nc.dram_tensor("out", [C, s], BF16, kind="ExternalOutput")

    with tile.TileContext(nc) as tc, ExitStack() as ctx:
        const = ctx.enter_context(tc.tile_pool(name="const", bufs=1))
        stream = ctx.enter_context(tc.tile_pool(name="stream", bufs=STREAM_BUFS))
        outp = ctx.enter_context(tc.tile_pool(name="outp", bufs=4))
        attn = ctx.enter_context(tc.tile_pool(name="attn", bufs=1))
        scr = ctx.enter_context(tc.tile_pool(name="scr", bufs=2))
        psA = ctx.enter_context(tc.tile_pool(name="psA", bufs=3, space="PSUM"))
        psM = ctx.enter_context(tc.tile_pool(name="psM", bufs=5, space="PSUM"))
        dramp = ctx.enter_context(tc.tile_pool(name="dramp", bufs=1, space="DRAM"))

        # ---- constant loads (off critical path) ----
        ident = const.tile([128, 128], f32, tag="ident", name="ident")
        nc.scalar.dma_start(out=ident[:], in_=id_d[:])
        wq2 = const.tile([2 * U + 1, dd], f32, tag="wq2", name="wq2")
        nc.scalar.dma_start(out=wq2[:], in_=wq2_d[:])
        wk2 = const.tile([2 * U + 1, dd], f32, tag="wk2", name="wk2")
        nc.scalar.dma_start(out=wk2[:], in_=wk2_d[:])
        wc_sb = []
        for oi in range(2):
            t = const.tile([128, 4 * C], f32, tag=f"wc{oi}", name=f"wc{oi}")
            for m in range(4):
                nc.scalar.dma_start(
                    out=t[:, m * C : (m + 1) * C],
                    in_=wc_d[m, oi * 128 : (oi + 1) * 128, :],
                )
            wc_sb.append(t)
        bc_sb = []
        for oi in range(2):
            t = const.tile([128, 4], f32, tag=f"bc{oi}", name=f"bc{oi}")
            nc.scalar.dma_start(out=t[:], in_=bcT_d[oi * 128 : (oi + 1) * 128, :])
            bc_sb.append(t)

        # persistent attention-state tiles
        praw = [
            attn.tile([128, 4 * U], f32, tag=f"praw{ci}", name=f"praw{ci}")
            for ci in range(2)
        ]
        pooled = [
            attn.tile([128, 4 * 2 * U], f32, tag=f"pool{k}", name=f"pool{k}")
            for k in range(2)
        ]
        ptaug = [
            attn.tile([2 * U + 1, C], f32, tag=f"pt{m}", name=f"pt{m}")
            for m in range(4)
        ]
        qc = [attn.tile([128, dd], f32, tag=f"qc{k}", name=f"qc{k}") for k in range(2)]
        kcs = [
            [attn.tile([128, dd], f32, tag=f"kc{m}_{k}", name=f"kc{m}_{k}") for k in range(2)]
            for m in range(4)
        ]
        xc = [
            attn.tile([128, max(ncache, 1) * uw], BF16, tag=f"xcache{ci}", name=f"xcache{ci}")
            for ci in range(2)
        ]
        wt_sb = [
            attn.tile([128, 4 * C], mm_dt, tag=f"wt{ci}", name=f"wt{ci}")
            for ci in range(2)
        ]
        wtb_sb = [
            attn.tile([128, 4 * C], BF16, tag=f"wtb{ci}", name=f"wtb{ci}")
            for ci in range(2)
        ]
        for m in range(4):
            nc.vector.memset(ptaug[m][:], 1.0)

        cc_in = [
            dramp.tile([128, 4 * U], f32, tag=f"cc_in{ci}", name=f"cc_in{ci}")
            for ci in range(2)
        ]
        cc_out = [
            dramp.tile([256, 4 * U], f32, tag=f"cc_out{ci}", name=f"cc_out{ci}")
            for ci in range(2)
        ]

        def chain_head(k):
            """Readback + transposes + q/k matmuls + psum->sbuf copies for
            channel-half k. PE + ACT only (no DVE: pass-1 reduces own DVE)."""
            for hh in range(2):
                eng = nc.scalar if hh == 0 else nc.gpsimd
                for m in range(4):
                    eng.dma_start(
                        out=pooled[k][:, m * 2 * U + hh * U : m * 2 * U + (hh + 1) * U],
                        in_=cc_out[k][
                            hh * 128 : (hh + 1) * 128, m * U : (m + 1) * U
                        ],
                    )
            for m in range(4):
                # whole [128, 2U] m-block in one transpose: PSUM result and
                # the ptaug copy both start at partition 0 (walrus requires
                # PSUMPartition == 0, and engines cannot shift partitions)
                pst = psA.tile([2 * U, 128], f32, tag="att", name="att")
                nc.tensor.transpose(
                    pst[:],
                    pooled[k][:, m * 2 * U : (m + 1) * 2 * U],
                    ident[:],
                )
                nc.scalar.activation(
                    ptaug[m][0 : 2 * U, k * 128 : (k + 1) * 128],
                    pst[:],
                    AF.Copy,
                )
            psq = psA.tile([128, dd], f32, tag="att", name="att")
            nc.tensor.matmul(
                psq[:], lhsT=ptaug[0][:, k * 128 : (k + 1) * 128], rhs=wq2[:],
                start=True, stop=True,
            )
            nc.scalar.activation(qc[k][:], psq[:], AF.Copy)
            for m in range(4):
                psk = psA.tile([128, dd], f32, tag="att", name="att")
                nc.tensor.matmul(
                    psk[:], lhsT=ptaug[m][:, k * 128 : (k + 1) * 128], rhs=wk2[:],
                    start=True, stop=True,
                )
                nc.scalar.activation(kcs[m][k][:], psk[:], AF.Copy)

        def chain_tail(k):
            """Logits + softmax + scaled weights for output-half oi=k.
            DVE-heavy; emitted only after all pass-1 reduces."""
            lg = attn.tile([128, 4], f32, tag=f"lg{k}", name=f"lg{k}")
            for m in range(4):
                sc = scr.tile([128, dd], f32, tag="ttr", name="ttr")
                nc.vector.tensor_mul(sc[:], qc[k][:], kcs[m][k][:])
                nc.vector.reduce_sum(out=lg[:, m : m + 1], in_=sc[:], axis=AX)
            # logits are O(1e-4) here (weights scaled by 0.02), so exp() is
            # safe without the max-subtraction step
            ex = attn.tile([128, 4], f32, tag=f"ex{k}", name=f"ex{k}")
            nc.scalar.activation(ex[:], lg[:], AF.Exp)
            sm = attn.tile([128, 1], f32, tag=f"sm{k}", name=f"sm{k}")
            nc.vector.reduce_sum(out=sm[:], in_=ex[:], axis=AX)
            rc = attn.tile([128, 1], f32, tag=f"rc{k}", name=f"rc{k}")
            nc.vector.reciprocal(out=rc[:], in_=sm[:])
            at = attn.tile([128, 4], f32, tag=f"a{k}", name=f"a{k}")
            nc.vector.tensor_scalar_mul(out=at[:], in0=ex[:], scalar1=rc[:])
            # weff = a * wc rows (oi = k), effective bias
            weff = attn.tile([128, 4 * C], f32, tag=f"weff{k}", name=f"weff{k}")
            for m in range(4):
                nc.vector.tensor_scalar_mul(
                    out=weff[:, m * C : (m + 1) * C],
                    in0=wc_sb[k][:, m * C : (m + 1) * C],
                    scalar1=at[:, m : m + 1],
                )
            bt = scr.tile([128, 4], f32, tag="btmp", name="btmp")
            be = attn.tile([128, 1], f32, tag=f"beff{k}", name=f"beff{k}")
            nc.vector.tensor_mul(bt[:], at[:], bc_sb[k][:])
            nc.vector.reduce_sum(out=be[:], in_=bt[:], axis=AX)
            # wt[ci][:, m*C + k*128 : +128] = weff[:, m*C + ci*128]^T
            for m in range(4):
                for ci in range(2):
                    psw = psA.tile([128, 128], f32, tag="att", name="att")
                    nc.tensor.transpose(
                        psw[:],
                        weff[:, m * C + ci * 128 : m * C + (ci + 1) * 128],
                        ident[:],
                    )
                    nc.vector.tensor_copy(
                        wt_sb[ci][:, m * C + k * 128 : m * C + (k + 1) * 128],
                        psw[:].bitcast(mm_dt),
                    )
                    nc.scalar.activation(
                        wtb_sb[ci][:, m * C + k * 128 : m * C + (k + 1) * 128],
                        psw[:],
                        AF.Copy,
                    )
            return be

        # ---- pass 1: stream ci=0 units, then ci=1 units ----
        for ci in range(2):
            for u in range(U):
                t = stream.tile([128, uw], f32, tag="x1", name="x1")
                nc.sync.dma_start(out=t[:], in_=xh[ci][:, u * uw : (u + 1) * uw])
                for m in range(4):
                    nc.vector.reduce_sum(
                        out=praw[ci][:, m * U + u : m * U + u + 1],
                        in_=t[:, m * nw : (m + 1) * nw],
                        axis=AX,
                    )
                if u >= ucache0:
                    nc.scalar.activation(
                        xc[ci][:, (u - ucache0) * uw : (u - ucache0 + 1) * uw],
                        t[:],
                        AF.Copy,
                    )
            nc.sync.dma_start(out=cc_in[ci][:], in_=praw[ci][:])
            nc.gpsimd.collective_compute(
                "AllGather",
                mybir.AluOpType.bypass,
                replica_groups=[[0, 1], [2, 3], [4, 5], [6, 7]],
                ins=[cc_in[ci].opt()],
                outs=[cc_out[ci].opt()],
            )
            if ci == 0:
                chain_head(0)

        beff0 = chain_tail(0)
        chain_head(1)

        # ---- pass 2 ----
        def mm_group(units, oi, beff, cached):
            """One LDW-group: the oi-half chunks for a list of units."""
            pss = [psM.tile([128, nw], f32, tag="ps", name="ps") for _ in units]
            for m in range(4):
                for ci in range(2):
                    for i, u in enumerate(units):
                        if cached:
                            rhs = xc[ci][
                                :,
                                (u - ucache0) * uw + m * nw : (u - ucache0) * uw + (m + 1) * nw,
                            ]
                            lhsT = wtb_sb[ci][:, m * C + oi * 128 : m * C + (oi + 1) * 128]
                        else:
                            rhs = xt[(ci, u)][:, m * nw : (m + 1) * nw]
                            lhsT = wt_sb[ci][:, m * C + oi * 128 : m * C + (oi + 1) * 128]
                        nc.tensor.matmul(
                            pss[i][:], lhsT=lhsT, rhs=rhs,
                            start=(m == 0 and ci == 0),
                            stop=(m == 3 and ci == 1),
                        )
            ot = outp.tile([128, len(units) * nw], BF16, tag="ot", name="ot")
            for i, u in enumerate(units):
                nc.vector.tensor_scalar_add(
                    out=ot[:, i * nw : (i + 1) * nw], in0=pss[i][:], scalar1=beff[:]
                )
            nc.scalar.dma_start(
                out=out_d[oi * 128 : (oi + 1) * 128, units[0] * nw : units[0] * nw + len(units) * nw],
                in_=ot[:],
            )

        cu = list(range(ucache0, U))
        cpairs = [cu[i : i + 2] for i in range(0, len(cu), 2)]
        ru = list(range(ucache0))
        rpairs = [ru[i : i + 2] for i in range(0, len(ru), 2)]

        # part 1: some cached-unit oi=0 groups bridge the collective +
        # chain latency; the rest become part-2 fillers
        n_p1 = min(4, len(cpairs))
        for pair in cpairs[:n_p1]:
            mm_group(pair, 0, beff0, cached=True)

        beff1 = chain_tail(1)

        # part 2: re-read pairs (both oi) with the remaining cached chunks
        # (leftover oi=0 groups, then all oi=1 groups) interleaved as fillers
        fillers = [(p, 0) for p in cpairs[n_p1:]] + [(p, 1) for p in cpairs]
        xt = {}
        for j, pair in enumerate(rpairs):
            for u in pair:
                for ci in range(2):
                    t = stream.tile([128, uw], mm_dt, tag="x1", name="x1")
                    nc.sync.dma_start(
                        out=t[:],
                        in_=xh[ci][:, u * uw : (u + 1) * uw].bitcast(mm_dt),
                    )
                    xt[(ci, u)] = t
            mm_group(pair, 0, beff0, cached=False)
            mm_group(pair, 1, beff1, cached=False)
            if fillers:
                fp, foi = fillers.pop(0)
                mm_group(fp, foi, beff0 if foi == 0 else beff1, cached=True)
        for fp, foi in fillers:
            mm_group(fp, foi, beff0 if foi == 0 else beff1, cached=True)
    return nc


_CACHED = {}
LAST_RESULTS = None


def _build(wh=WH, dhalf=DHALF, mm_dt=None):
    if mm_dt is None:
        mm_dt = MM_DT
    key = (wh, dhalf, mm_dt)
    if key not in _CACHED:
        nc = bacc.Bacc(
            "TRN2",
            target_bir_lowering=False,
            debug=False,
            enable_asserts=False,
            num_devices=NCORES,
        )
        _emit_program(nc, wh=wh, dhalf=dhalf, mm_dt=mm_dt)
        nc.compile()
        _CACHED[key] = nc
    return _CACHED[key]


def _host_weights(Wq, bq, Wk, bk, bc, wh, dhalf):
    """Fold pooling mean + logit scale into augmented [2U+1, D] q/k weights
    with one row per (d-half, unit) pooling partial and a trailing ones-row
    for the bias."""
    nw, U, _ = _cfg(wh, dhalf)
    d = 2 * dhalf
    scale_q = 1.0 / (wh * np.sqrt(np.float32(d)))
    wqT = (Wq * scale_q).T  # [d, d] row = source dim
    wkT = (Wk / wh).T
    units_per_d = wh // nw
    wq2 = np.zeros((2 * U + 1, d), np.float32)
    wk2 = np.zeros((2 * U + 1, d), np.float32)
    for hh in range(2):
        for u in range(U):
            dg = hh * dhalf + u // units_per_d
            wq2[hh * U + u] = wqT[dg]
            wk2[hh * U + u] = wkT[dg]
    wq2[2 * U] = bq / np.sqrt(np.float32(d))
    wk2[2 * U] = bk
    bcT = np.ascontiguousarray(bc.T).astype(np.float32)
    ident = np.eye(128, dtype=np.float32)
    return wq2, wk2, bcT, ident


def _core_xh(ms, b, h, wh, dhalf):
    """Repack core (b, h)'s shard into the two [128, U*4*nw] unit tensors."""
    nw, U, _ = _cfg(wh, dhalf)
    s = dhalf * wh
    res = []
    for ci in range(2):
        blocks = [
            ms[m][b, ci * 128 : (ci + 1) * 128, h * dhalf : (h + 1) * dhalf]
            .reshape(128, U, nw)
            for m in range(4)
        ]
        arr = np.stack(blocks, axis=2)  # [128, U, 4, nw]
        res.append(np.ascontiguousarray(arr).reshape(128, U * 4 * nw))
    return res


def kernel(m1, m2, m3, m4, Wq, bq, Wk, bk, Wc, bc, **run_kwargs):
    ms = [np.asarray(x, dtype=np.float32) for x in (m1, m2, m3, m4)]
    Wq, bq, Wk, bk, Wc, bc = (
        np.asarray(x, dtype=np.float32) for x in (Wq, bq, Wk, bk, Wc, bc)
    )
    nc = _build()
    wq2, wk2, bcT, ident = _host_weights(Wq, bq, Wk, bk, bc, WH, DHALF)
    in_maps = []
    for p in range(NCORES):
        b, h = divmod(p, 2)
        x0, x1 = _core_xh(ms, b, h, WH, DHALF)
        im = dict(xh0=x0, xh1=x1, wq2=wq2, wk2=wk2, wc=Wc, bcT=bcT, ident=ident)
        in_maps.append(im)
    global LAST_RESULTS
    res = run_bass_kernel_spmd(
        nc, in_maps, core_ids=list(range(NCORES)), **run_kwargs
    )
    LAST_RESULTS = res
    out = np.empty((B, C, D, W, H), np.float32)
    for p in range(NCORES):
        b, h = divmod(p, 2)
        out[b, :, h * DHALF : (h + 1) * DHALF] = (
            res.results[p]["out"].astype(np.float32).reshape(C, DHALF, W, H)
        )
    return out
